# revision 1
# baseline (speedup 1.0000x reference)
"""Trainium2 Bass kernel for nn_AugmentedAffinityContrastive.

Same math as v3 (fp8 ship, bf16 compute, per-plane product groups split
DVE/GpSimd, add tree, fused reduce), plus:

 - the final multiply-reduce is split: DVE does prod = e*G (bf16 2x-mode
   tensor_tensor, ~0.9us) and the otherwise-idle Scalar engine does the
   fused sum (activation Copy with accum_out), taking the 1x-rate
   reduction off the critical DVE path;
 - the whole 16-plane pass can repeat R times via a HARDWARE Fori loop
   (first pass peeled; in-loop semaphore thresholds live in registers
   bumped by constants), so one dispatch runs R full executions with a
   tiny program -- stable timing, no instruction-fetch pressure.

Engine budget per plane (HW-microbenched): DVE ~15.2us (2 grouped
products 3.3+2.4, add tree ~8.6, final prod 0.9), GpSimd ~14.2us
(2 grouped products of the odd-dx offsets), ACT ~12.7us (fp8->bf16
conversions + reduce).
"""

import numpy as np

import bass_rust

OFFS = [[0, -1], [-1, 0], [-1, -1], [0, -2], [-2, 0], [-2, -2], [0, -3], [-3, 0], [-3, -3]]
SIGMA = 1.2
B, E, H, W = 4, 32, 512, 512
NCORES = 8
PLANES = B * E
PPC = PLANES // NCORES  # 16 planes per core
P = 128
HB = H // P             # 4 row chunks per partition; h = p*HB + hb
HALO = 7                # tf halo rows per partition: 4p-3 .. 4p+3
WH = W + 4              # w-halo pad of 4 -> w' = 4 + w
TFN = HALO * WH         # 3612
X = HB * W              # 2048

GORDER = [(3, 0), (2, 0), (1, 0), (2, 2), (0, 2), (0, 3), (0, 1), (3, 3), (1, 1)]
GIDX = [OFFS.index([-dy, -dx]) for dy, dx in GORDER]
# DVE takes 7 of 9 offsets (unaligned odd-dx reads cost nothing extra on
# HW) so the GpSimd helper stays far below the DVE critical path and its
# semaphore handoff never blocks.
GROUPS = [  # (first slot, count, base elems, stride elems, engine)
    (0, 3, 4, WH, "dve"),
    (3, 2, 518, 2 * WH, "dve"),
    (5, 2, 1549, 2, "gp"),
    (7, 1, 1, 1, "gp"),
    (8, 1, 1035, 1, "dve"),
]

_CACHE = {}


def _blur_axis_np(x, k, r, axis):
    pad = [(0, 0)] * x.ndim
    pad[axis] = (r, r)
    xp = np.pad(x, pad, mode='edge')
    n = x.shape[axis]

    def sl(i):
        idx = [slice(None)] * x.ndim
        idx[axis] = slice(i, i + n)
        return xp[tuple(idx)]

    out = (k[0] * sl(0)).astype(np.float32)
    for i in range(1, 2 * r + 1):
        out = out + k[i] * sl(i)
    return out


def _host_cmaps(raw, mask):
    r = int(4.0 * SIGMA + 0.5)
    t = np.arange(-r, r + 1)
    k = np.exp(-0.5 * (t / SIGMA) ** 2)
    k = (k / k.sum()).astype(np.float32)

    x = raw[0].astype(np.float32)
    x = _blur_axis_np(_blur_axis_np(x, k, r, 1), k, r, 2)

    cs = []
    for i, off in enumerate(OFFS):
        rolled = np.roll(x, (-off[0], -off[1]), axis=(-2, -1))
        d = np.sqrt(((x - rolled) ** 2).sum(0))
        a = d / d.max()
        a = np.clip(a, 0.0, 1.0)
        a = a - a.min()
        a = a / a.max()
        cs.append(mask[0, i] * (np.float32(0.5) - a))
    return np.ascontiguousarray(np.stack(cs).astype(np.float32))


def _fp8():
    import concourse.mybir as mybir
    return mybir.dt.np(mybir.dt.float8e4)


def _bf16():
    import concourse.mybir as mybir
    return mybir.dt.np(mybir.dt.bfloat16)


def _prepare(embeds, tf_embeds, raw, mask):
    fp8 = _fp8()
    bf16 = _bf16()
    c = _host_cmaps(np.asarray(raw, np.float32), np.asarray(mask, np.float32))
    csum = c.sum(dtype=np.float64)

    cl = np.ascontiguousarray(
        c.reshape(9, P, HB, W).transpose(1, 0, 2, 3)[:, GIDX].astype(bf16))

    e128 = np.asarray(embeds, np.float32).reshape(PLANES, H, W).astype(fp8)
    t128 = np.asarray(tf_embeds, np.float32).reshape(PLANES, H, W).astype(fp8)

    padded = np.concatenate([t128[:, -3:], t128], axis=1)
    s0, s1, s2 = padded.strides
    rowv = np.lib.stride_tricks.as_strided(
        padded, shape=(PLANES, P, HALO, W), strides=(s0, HB * s1, s1, s2))
    tfh = np.ascontiguousarray(
        np.concatenate([rowv[..., W - 4:], rowv], axis=-1))  # [PLANES,P,7,516]

    in_maps = [
        {
            "e_sh": np.ascontiguousarray(e128[ci * PPC:(ci + 1) * PPC]),
            "tf_sh": np.ascontiguousarray(tfh[ci * PPC:(ci + 1) * PPC]),
            "cmap": cl,
        }
        for ci in range(NCORES)
    ]
    return in_maps, csum


def _build_bass(nloop=1):
    import concourse.bass as bass
    import concourse.mybir as mybir
    from contextlib import ExitStack

    bf = mybir.dt.bfloat16
    f8 = mybir.dt.float8e4
    f32 = mybir.dt.float32
    MUL = mybir.AluOpType.mult
    ADD = mybir.AluOpType.add
    COPY = mybir.ActivationFunctionType.Copy

    NSLOT = 2    # tmp / prod slots
    NIO = 4      # staging + bf16 io slots (deeper: relaxes the
                 # DVE->ACT-conv->GpSimd critical cycle to a 4-plane
                 # period; must divide PPC so loop-body slots are static)
    NQ = PPC * nloop

    nc = bass.Bass(detect_race_conditions=False)
    e_in = nc.dram_tensor("e_sh", [PPC, H, W], f8, kind="ExternalInput")
    tf_in = nc.dram_tensor("tf_sh", [PPC, P, HALO, WH], f8, kind="ExternalInput")
    c_in = nc.dram_tensor("cmap", [P, 9, HB, W], bf, kind="ExternalInput")
    parts_out = nc.dram_tensor("partials", [P, PPC], f32, kind="ExternalOutput")

    with (
        ExitStack() as stack,
        nc.sbuf_tensor([P, 9 * X], bf) as ct_s,
        nc.sbuf_tensor([P, NIO * X], f8) as e8_s,
        nc.sbuf_tensor([P, NIO * X], bf) as e_s,
        nc.sbuf_tensor([P, NIO * TFN], f8) as tf8_s,
        nc.sbuf_tensor([P, NIO * TFN], bf) as tf_s,
        nc.sbuf_tensor([P, NSLOT * 9 * X], bf) as tmp_s,
        nc.sbuf_tensor([P, 2 * X], bf) as v_s,
        nc.sbuf_tensor([P, X], bf) as w_s,
        nc.sbuf_tensor([P, X], bf) as g_s,
        nc.sbuf_tensor([P, X], bf) as junk_s,
        nc.sbuf_tensor([P, NSLOT * X], bf) as prod_s,
        nc.sbuf_tensor([P, PPC], f32) as parts_s,
        nc.semaphore() as csem,
        nc.semaphore() as osem,
        nc.semaphore() as vsem,   # DVE plane counter (inc at final prod)
        nc.semaphore() as gsem,   # GpSimd product counter (2/plane)
        nc.semaphore() as asem,   # ACT conversion counter (2/plane)
        nc.semaphore() as rsem,   # ACT reduce counter (1/plane, lag-1)
        nc.Block() as block,
    ):
        psem = [stack.enter_context(nc.semaphore(name=f"psem{q}"))
                for q in range(PPC)]

        ct_v = ct_s[:].rearrange("p (g hb w) -> p g hb w", g=9, w=W)
        e_v = e_s[:].rearrange("p (s x) -> p s x", s=NIO)
        e8_v = e8_s[:].rearrange("p (s x) -> p s x", s=NIO)
        tf8_v = tf8_s[:].rearrange("p (s y) -> p s y", s=NIO)
        tf_v = tf_s[:].rearrange("p (s y) -> p s y", s=NIO)
        tmp_v = tmp_s[:].rearrange("p (s g hb w) -> p s g hb w", s=NSLOT, g=9, w=W)
        v_v = v_s[:].rearrange("p (a x) -> p a x", a=2)
        prod_v = prod_s[:].rearrange("p (s x) -> p s x", s=NSLOT)
        tf_flat = tf_s[:]

        def tf_ap(sl, base, stride, k):
            return bass_rust.AP(
                tensor=tf_flat.tensor,
                ap=[list(tf_flat.ap[0]), [stride, k], [WH, HB], [1, W]],
                offset=sl * TFN + base,
            )

        # ---- sync: all DMAs ------------------------------------------------
        @block.sync
        def _(sync):
            def loads(Q, r_as=None):
                q = Q % PPC
                sl = Q % NIO
                if r_as is not None:
                    sync.wait_ge(asem, r_as)          # convs of plane Q-4 done
                    sync.reg_alu(r_as, r_as, 2, ADD)
                elif Q >= NIO:
                    sync.wait_ge(asem, 2 * Q - 6)     # = 2*(Q-4) + 2
                sync.dma_start(
                    tf8_v[:, sl],
                    tf_in[q].rearrange("p c w -> p (c w)"),
                ).then_inc(psem[q], 16)
                sync.dma_start(
                    e8_v[:, sl],
                    e_in[q].rearrange("(p hb) w -> p (hb w)", hb=HB),
                ).then_inc(psem[q], 16)

            sync.dma_start(ct_v, c_in[:]).then_inc(csem, 16)
            for Q in range(PPC):                      # peeled first pass
                loads(Q)
            if nloop > 1:
                r_as = sync.alloc_register("r_as")
                sync.reg_mov(r_as, 2 * PPC - 6)
                with sync.Fori(0, nloop - 1):
                    for q in range(PPC):
                        loads(PPC + q, r_as)          # Q%PPC / Q%NIO == q's
            sync.wait_ge(rsem, NQ)                    # all ACT reduces done
            sync.dma_start(parts_out[:], parts_s[:]).then_inc(osem, 16)
            sync.wait_ge(osem, 16)

        # ---- ACT: fp8->bf16 conversions + lag-1 final reduce ---------------
        # body j: [convs for plane j] then [reduce for plane j-1]; the
        # reduce rides in ACT slack and DVE gates only on conversions
        # (asem) -- reduces are counted separately (rsem)
        def act_reduce(j):
            # `out` is a dump buffer; only the f32 accum_out matters
            return nc.scalar.activation(
                junk_s[:], prod_v[:, j % NSLOT], COPY,
                accum_out=parts_s[:, (j % PPC):(j % PPC) + 1],
            ).then_inc(rsem, 1)

        @block.scalar
        def _(scalar):
            def pass_plane(Q, regs=None):
                q = Q % PPC
                sl = Q % NIO
                if regs is None:
                    scalar.wait_ge(psem[q], 32)
                    if Q >= NIO:
                        scalar.wait_ge(vsem, Q - 3)   # DVE done with slot Q-4
                        scalar.wait_ge(gsem, 2 * Q - 6)
                else:
                    r_ps, r_vs1, r_vs2, r_gs = regs
                    scalar.wait_ge(psem[q], r_ps)
                    scalar.wait_ge(vsem, r_vs1)
                    scalar.reg_alu(r_vs1, r_vs1, 1, ADD)
                    scalar.wait_ge(gsem, r_gs)
                    scalar.reg_alu(r_gs, r_gs, 2, ADD)
                nc.scalar.copy(tf_v[:, sl], tf8_v[:, sl]).then_inc(asem, 1)
                nc.scalar.copy(e_v[:, sl], e8_v[:, sl]).then_inc(asem, 1)
                if Q >= 1:
                    if regs is None:
                        scalar.wait_ge(vsem, Q)       # DVE prod(Q-1) landed
                    else:
                        scalar.wait_ge(vsem, r_vs2)
                        scalar.reg_alu(r_vs2, r_vs2, 1, ADD)
                    act_reduce(Q - 1)

            for Q in range(PPC):                      # peeled first pass
                pass_plane(Q)
            if nloop > 1:
                r_ps = scalar.alloc_register("r_ps")
                r_vs1 = scalar.alloc_register("r_vs1")
                r_vs2 = scalar.alloc_register("r_vs2")
                r_gs = scalar.alloc_register("r_gs")
                scalar.reg_mov(r_ps, 64)
                scalar.reg_mov(r_vs1, PPC - 3)
                scalar.reg_mov(r_vs2, PPC)
                scalar.reg_mov(r_gs, 2 * PPC - 6)
                with scalar.Fori(0, nloop - 1):
                    for q in range(PPC):
                        pass_plane(PPC + q, (r_ps, r_vs1, r_vs2, r_gs))
                    scalar.reg_alu(r_ps, r_ps, 32, ADD)
            # final lagging reduce for the last plane
            scalar.wait_ge(vsem, NQ)
            act_reduce(NQ - 1)

        # ---- GpSimd: odd-dx products ---------------------------------------
        @block.gpsimd
        def _(gpsimd):
            def pass_plane(Q, regs=None):
                sl = Q % NSLOT
                slio = Q % NIO
                if regs is None:
                    gpsimd.wait_ge(asem, 2 * Q + 1)   # tf-conv of plane Q
                    if Q >= NSLOT:
                        gpsimd.wait_ge(vsem, Q - 1)   # tmp slot consumed
                else:
                    r_ga, r_gv = regs
                    gpsimd.wait_ge(asem, r_ga)
                    gpsimd.reg_alu(r_ga, r_ga, 2, ADD)
                    gpsimd.wait_ge(vsem, r_gv)
                    gpsimd.reg_alu(r_gv, r_gv, 1, ADD)
                for g0, k, base, stride, eng in GROUPS:
                    if eng != "gp":
                        continue
                    nc.gpsimd.tensor_tensor(
                        tmp_v[:, sl, g0:g0 + k],
                        ct_v[:, g0:g0 + k],
                        tf_ap(slio, base, stride, k),
                        MUL,
                    ).then_inc(gsem, 1)

            gpsimd.wait_ge(csem, 16)
            for Q in range(PPC):
                pass_plane(Q)
            if nloop > 1:
                r_ga = gpsimd.alloc_register("r_ga")
                r_gv = gpsimd.alloc_register("r_gv")
                gpsimd.reg_mov(r_ga, 2 * PPC + 1)
                gpsimd.reg_mov(r_gv, PPC - 1)
                with gpsimd.Fori(0, nloop - 1):
                    for q in range(PPC):
                        pass_plane(PPC + q, (r_ga, r_gv))

        # ---- DVE: even-dx products + add tree + final product --------------
        @block.vector
        def _(vector):
            def pass_plane(Q, regs=None):
                sl = Q % NSLOT
                slio = Q % NIO
                if regs is None:
                    vector.wait_ge(asem, 2 * Q + 2)   # tf+e convs of plane Q
                else:
                    r_da, r_dg, r_dr = regs
                    vector.wait_ge(asem, r_da)
                    vector.reg_alu(r_da, r_da, 2, ADD)
                for g0, k, base, stride, eng in GROUPS:
                    if eng != "dve":
                        continue
                    nc.vector.tensor_tensor(
                        tmp_v[:, sl, g0:g0 + k],
                        ct_v[:, g0:g0 + k],
                        tf_ap(slio, base, stride, k),
                        MUL,
                    )
                if regs is None:
                    vector.wait_ge(gsem, 2 * (Q + 1))
                else:
                    vector.wait_ge(gsem, r_dg)
                    vector.reg_alu(r_dg, r_dg, 2, ADD)
                t8 = tmp_v[:, sl].rearrange("p g hb w -> p (g hb w)")
                # L1 in-place: tmp[0:4] += tmp[4:8] (same-index elementwise;
                # the read of each element precedes its write in the pipe)
                nc.vector.tensor_tensor(
                    t8[:, 0:4 * X], t8[:, 0:4 * X], t8[:, 4 * X:8 * X], ADD)
                nc.vector.tensor_tensor(
                    v_v[:].rearrange("p a x -> p (a x)"),
                    t8[:, 0:2 * X], t8[:, 2 * X:4 * X], ADD)
                nc.vector.tensor_tensor(w_s[:], v_v[:, 0], v_v[:, 1], ADD)
                nc.vector.tensor_tensor(g_s[:], w_s[:], t8[:, 8 * X:9 * X], ADD)
                if regs is None:
                    if Q >= NSLOT:
                        vector.wait_ge(rsem, Q - 1)   # reduce(Q-2) done
                else:
                    vector.wait_ge(rsem, r_dr)
                    vector.reg_alu(r_dr, r_dr, 1, ADD)
                nc.vector.tensor_tensor(
                    prod_v[:, sl], e_v[:, slio], g_s[:], MUL,
                ).then_inc(vsem, 1)

            vector.wait_ge(csem, 16)
            for Q in range(PPC):
                pass_plane(Q)
            if nloop > 1:
                r_da = vector.alloc_register("r_da")
                r_dg = vector.alloc_register("r_dg")
                r_dr = vector.alloc_register("r_dr")
                vector.reg_mov(r_da, 2 * PPC + 2)
                vector.reg_mov(r_dg, 2 * PPC + 2)
                vector.reg_mov(r_dr, PPC - 1)
                with vector.Fori(0, nloop - 1):
                    for q in range(PPC):
                        pass_plane(PPC + q, (r_da, r_dg, r_dr))
    return nc

def _get_nc(nloop=1):
    key = f"nc{nloop}"
    if key not in _CACHE:
        _CACHE[key] = _build_bass(nloop)
    return _CACHE[key]


def _make_runner(nc, in_maps):
    import time
    import jax
    import concourse.mybir as mybir
    from concourse import bass2jax
    from jax.sharding import Mesh, PartitionSpec, NamedSharding
    from jax.experimental.shard_map import shard_map

    pid = nc.partition_id_tensor.name if nc.partition_id_tensor else None
    in_names, out_names, out_avals, zeros = [], [], [], []
    for alloc in nc.m.functions[0].allocations:
        if type(alloc).__name__ != "MemoryLocationSet":
            continue
        name = alloc.memorylocations[0].name
        if alloc.kind == "ExternalInput":
            if name != pid:
                in_names.append(name)
        elif alloc.kind == "ExternalOutput":
            out_names.append(name)
            shape = tuple(alloc.tensor_shape)
            dt = mybir.dt.np(alloc.dtype)
            out_avals.append(jax.core.ShapedArray(shape, dt))
            zeros.append(np.zeros(shape, dt))
    n_params = len(in_names)
    all_names = in_names + out_names + ([pid] if pid else [])

    def _body(*args):
        ops = list(args)
        if pid:
            ops.append(bass2jax.partition_id_tensor())
        return tuple(bass2jax._bass_exec_p.bind(
            *ops, out_avals=tuple(out_avals), in_names=tuple(all_names),
            out_names=tuple(out_names), lowering_input_output_aliases=(),
            sim_require_finite=True, sim_require_nnan=True, nc=nc))

    devices = jax.devices()[:NCORES]
    mesh = Mesh(np.asarray(devices), ("core",))
    n_outs = len(out_names)
    sharded = jax.jit(
        shard_map(_body, mesh=mesh,
                  in_specs=(PartitionSpec("core"),) * (n_params + n_outs),
                  out_specs=(PartitionSpec("core"),) * n_outs,
                  check_rep=False),
        donate_argnums=tuple(range(n_params, n_params + n_outs)),
        keep_unused=True)
    sh = NamedSharding(mesh, PartitionSpec("core"))
    d_in = [jax.device_put(
                np.concatenate([np.asarray(m[k]) for m in in_maps], axis=0), sh)
            for k in in_names]
    cz = [np.concatenate([z] * NCORES, axis=0) for z in zeros]

    def run_once():
        dz = [jax.device_put(z, sh) for z in cz]
        for a in dz:
            a.block_until_ready()
        t0 = time.perf_counter()
        outs = sharded(*d_in, *dz)
        for o in outs:
            o.block_until_ready()
        return time.perf_counter() - t0
    return run_once


def benchmark(embeds, tf_embeds, raw, mask, iters=20, nloop=64):
    """Paired null/main timing of the hardware-looped program."""
    import concourse.bass as bass
    import concourse.mybir as mybir

    in_maps, _ = _prepare(embeds, tf_embeds, raw, mask)
    run_main = _make_runner(_get_nc(nloop), in_maps)

    f32 = mybir.dt.float32
    nc2 = bass.Bass()
    a_in = nc2.dram_tensor("a", [P, 16], f32, kind="ExternalInput")
    b_out = nc2.dram_tensor("b", [P, 16], f32, kind="ExternalOutput")
    with (nc2.sbuf_tensor([P, 16], f32) as t,
          nc2.semaphore() as s,
          nc2.semaphore() as o,
          nc2.Block() as blk):
        @blk.sync
        def _(sync):
            sync.dma_start(t[:], a_in[:]).then_inc(s, 16)
            sync.wait_ge(s, 16)
            sync.dma_start(b_out[:], t[:]).then_inc(o, 16)
            sync.wait_ge(o, 16)
    null_maps = [{"a": np.zeros((P, 16), np.float32)} for _ in range(NCORES)]
    run_null = _make_runner(nc2, null_maps)

    main_ts, null_ts = [], []
    for _ in range(iters):
        null_ts.append(run_null())
        main_ts.append(run_main())
    return main_ts, null_ts


def kernel(embeds, tf_embeds, raw, mask):
    from concourse.bass_utils import run_bass_kernel_spmd

    in_maps, csum = _prepare(embeds, tf_embeds, raw, mask)
    res = run_bass_kernel_spmd(
        _get_nc(1), in_maps, core_ids=list(range(NCORES)),
    )
    _CACHE["last_results"] = res

    s = np.float64(0.0)
    for om in res.results:
        s += om["partials"].astype(np.float64).sum()

    loss = (B * csum - s) / float(B * H * W)
    return np.asarray(loss, dtype=np.float32)



# revision 7
# speedup vs baseline: 1.1312x; 1.1312x over previous
"""Trainium2 Bass kernel for nn_AugmentedAffinityContrastive — v3.

Math: loss = (B*csum - S) / (B*H*W) with
  S = sum_planes sum_hw e ⊙ G,   G = sum_i c_i ⊙ t_shift_i
c maps are host-precomputed (batch-free, tiny); S is the device work.

v4 (contention-driven rebalance over v3):
 - GpSimd is DROPPED from the hot path: it shares its SBUF port with
   DVE (the POOL slot), and one GpSimd product measured +3.8us/plane of
   DVE slowdown (dve 8.9 -> dve+gp 12.8) — worse than doing the slot on
   DVE (+1.0us). DVE does all 9 products as three k=3 grouped
   tensor_tensors (rows s516 / cols s1 / diags s517, ~3.0us each)
   plus the final e*G multiply (1.1us);
 - single tf halo; odd/unaligned bases measured free on this HW;
 - TensorEngine absorbs the 9-way add tree: identity-weight matmuls
   accumulate tmp into PSUM (36 N=512 MMs, ~8.5us measured warm);
 - ACT: G-copy PSUM->SBUF + accumulating reduce (~4.7us pipelined).

Per-plane budget (measured): DVE 10.1, PE 8.5, ACT 4.7, DMA 4.2
-> ~10.3us/plane x 16 planes ~ 165us/core target.
"""

import numpy as np

import bass_rust

OFFS = [[0, -1], [-1, 0], [-1, -1], [0, -2], [-2, 0], [-2, -2], [0, -3], [-3, 0], [-3, -3]]
SIGMA = 1.2
B, E, H, W = 4, 32, 512, 512
NCORES = 8
PLANES = B * E
PPC = PLANES // NCORES  # 16 planes per core
P = 128
HB = H // P             # 4 row chunks per partition; h = p*HB + hb
HALO = 7                # tf halo rows per partition: 4p-3 .. 4p+3
WH = W + 4              # w-halo pad of 4 -> w' = 4 + w (halo A)
TFN = HALO * WH         # 3612 elems per halo
X = HB * W              # 2048
NIO = 4                 # io slots (must divide PPC)
NSLOT = 2               # tmp/psum/G/prod double-buffer

# slot offsets (o0,o1):  t_shift[h,w] = t[h+o0, w+o1]
# single halo; odd bases cost nothing on this HW (microbenched)
SLOT_OFFS = [(-3, 0), (-2, 0), (-1, 0), (0, -3), (0, -2), (0, -1),
             (-3, -3), (-2, -2), (-1, -1)]
GIDX = [OFFS.index([o0, o1]) for o0, o1 in SLOT_OFFS]
# product groups: (first slot, count, base elems, stride, engine)
#  base = (o0+3)*516 + 4+o1
GROUPS = [
    (0, 3, 4, 516, "dve"),          # (-3,0),(-2,0),(-1,0)
    (3, 3, 1549, 1, "dve"),         # (0,-3),(0,-2),(0,-1)
    (6, 3, 1, 517, "dve"),          # (-3,-3),(-2,-2),(-1,-1)
]
PE_GROUPS = [(0, 1, 2), (3, 4, 5), (6, 7, 8)]

_CACHE = {}


def _blur_axis_np(x, k, r, axis):
    pad = [(0, 0)] * x.ndim
    pad[axis] = (r, r)
    xp = np.pad(x, pad, mode='edge')
    n = x.shape[axis]

    def sl(i):
        idx = [slice(None)] * x.ndim
        idx[axis] = slice(i, i + n)
        return xp[tuple(idx)]

    out = (k[0] * sl(0)).astype(np.float32)
    for i in range(1, 2 * r + 1):
        out = out + k[i] * sl(i)
    return out


def _host_cmaps(raw, mask):
    r = int(4.0 * SIGMA + 0.5)
    t = np.arange(-r, r + 1)
    k = np.exp(-0.5 * (t / SIGMA) ** 2)
    k = (k / k.sum()).astype(np.float32)

    x = raw[0].astype(np.float32)
    x = _blur_axis_np(_blur_axis_np(x, k, r, 1), k, r, 2)

    cs = []
    for i, off in enumerate(OFFS):
        rolled = np.roll(x, (-off[0], -off[1]), axis=(-2, -1))
        d = np.sqrt(((x - rolled) ** 2).sum(0))
        a = d / d.max()
        a = np.clip(a, 0.0, 1.0)
        a = a - a.min()
        a = a / a.max()
        cs.append(mask[0, i] * (np.float32(0.5) - a))
    return np.ascontiguousarray(np.stack(cs).astype(np.float32))


def _bf16():
    import concourse.mybir as mybir
    return mybir.dt.np(mybir.dt.bfloat16)


def _prepare(embeds, tf_embeds, raw, mask):
    bf16 = _bf16()
    c = _host_cmaps(np.asarray(raw, np.float32), np.asarray(mask, np.float32))
    csum = c.sum(dtype=np.float64)

    cl = np.ascontiguousarray(
        c.reshape(9, P, HB, W).transpose(1, 0, 2, 3)[:, GIDX].astype(bf16))

    e128 = np.asarray(embeds, np.float32).reshape(PLANES, H, W).astype(bf16)
    t128 = np.asarray(tf_embeds, np.float32).reshape(PLANES, H, W).astype(bf16)

    padded = np.concatenate([t128[:, -3:], t128], axis=1)
    s0, s1, s2 = padded.strides
    rowv = np.lib.stride_tricks.as_strided(
        padded, shape=(PLANES, P, HALO, W), strides=(s0, HB * s1, s1, s2))
    tfh = np.ascontiguousarray(
        np.concatenate([rowv[..., W - 4:], rowv], axis=-1))  # [PLANES,P,7,516]

    ident = np.eye(P, dtype=bf16)

    in_maps = [
        {
            "e_sh": np.ascontiguousarray(e128[ci * PPC:(ci + 1) * PPC]),
            "tf_sh": np.ascontiguousarray(tfh[ci * PPC:(ci + 1) * PPC]),
            "cmap": cl,
            "ident": ident,
        }
        for ci in range(NCORES)
    ]
    return in_maps, csum


def _build_bass(nloop=1):
    import concourse.bass as bass
    import concourse.mybir as mybir
    from contextlib import ExitStack

    bf = mybir.dt.bfloat16
    f32 = mybir.dt.float32
    MUL = mybir.AluOpType.mult
    ADD = mybir.AluOpType.add
    COPY = mybir.ActivationFunctionType.Copy

    NQ = PPC * nloop

    nc = bass.Bass(detect_race_conditions=False)
    e_in = nc.dram_tensor("e_sh", [PPC, H, W], bf, kind="ExternalInput")
    tf_in = nc.dram_tensor("tf_sh", [PPC, P, HALO, WH], bf, kind="ExternalInput")
    c_in = nc.dram_tensor("cmap", [P, 9, HB, W], bf, kind="ExternalInput")
    i_in = nc.dram_tensor("ident", [P, P], bf, kind="ExternalInput")
    parts_out = nc.dram_tensor("partials", [P, PPC], f32, kind="ExternalOutput")

    with (
        ExitStack() as stack,
        nc.sbuf_tensor([P, 9 * X], bf) as ct_s,
        nc.sbuf_tensor([P, NIO * X], bf) as e_s,
        nc.sbuf_tensor([P, NIO * TFN], bf) as tf_s,
        nc.sbuf_tensor([P, NSLOT * 9 * X], bf) as tmp_s,
        nc.sbuf_tensor([P, NSLOT * X], bf) as g_s,
        nc.sbuf_tensor([P, NSLOT * X], bf) as prod_s,
        nc.sbuf_tensor([P, X], bf) as junk_s,
        nc.sbuf_tensor([P, P], bf) as id_s,
        nc.sbuf_tensor([P, PPC], f32) as parts_s,
        nc.psum_tensor([P, 2 * X], f32) as psum_t,            # 2 sets x 4 banks
        nc.semaphore() as csem,    # cmap+ident DMA
        nc.semaphore() as esem,    # e DMA (16/plane; dsem[q] is tf-only so
                                   # its >=16 wait proves the halo landed)
        nc.semaphore() as osem,    # output DMA
        nc.semaphore() as vsem,    # DVE product groups (3/plane)
        nc.semaphore() as pesem,   # PE matmul groups (3/plane)
        nc.semaphore() as gcsem,   # ACT G-copy (1/plane)
        nc.semaphore() as msem,    # DVE final mul (1/plane)
        nc.semaphore() as rsem,    # ACT reduce (1/plane)
        nc.Block() as block,
    ):
        dsem = [stack.enter_context(nc.semaphore(name=f"dsem{q}"))
                for q in range(PPC)]

        ct_v = ct_s[:].rearrange("p (g hb w) -> p g hb w", g=9, w=W)
        e_v = e_s[:].rearrange("p (s x) -> p s x", s=NIO)
        tmp_v = tmp_s[:].rearrange("p (s g hb w) -> p s g hb w", s=NSLOT, g=9, w=W)
        g_v = g_s[:].rearrange("p (s x) -> p s x", s=NSLOT)
        prod_v = prod_s[:].rearrange("p (s x) -> p s x", s=NSLOT)
        psum_v = psum_t[:].rearrange("p (s b x) -> p s b x", s=NSLOT, b=HB)
        psum_f = psum_t[:].rearrange("p (s y) -> p s y", s=NSLOT)
        tf_flat = tf_s[:]

        def tf_ap(sl, base, stride, k):
            return bass_rust.AP(
                tensor=tf_flat.tensor,
                ap=[list(tf_flat.ap[0]), [stride, k], [WH, HB], [1, W]],
                offset=sl * TFN + base,
            )

        # ---- sync: all DMAs ------------------------------------------------
        @block.sync
        def _(sync):
            def loads(Q, regs=None):
                q = Q % PPC
                sl = Q % NIO
                if regs is None:
                    if Q >= NIO:
                        # tf slot reuse: products of plane Q-NIO done
                        sync.wait_ge(vsem, 3 * (Q - NIO) + 3)
                else:
                    r_v, r_m = regs
                    sync.wait_ge(vsem, r_v)
                    sync.reg_alu(r_v, r_v, 3, ADD)
                sync.dma_start(
                    tf_s[:, sl * TFN:(sl + 1) * TFN],
                    tf_in[q].rearrange("p c w -> p (c w)"),
                ).then_inc(dsem[q], 16)
                if regs is None:
                    if Q >= NIO:
                        sync.wait_ge(msem, Q - NIO + 1)  # e slot reuse
                else:
                    sync.wait_ge(msem, regs[1])
                    sync.reg_alu(regs[1], regs[1], 1, ADD)
                sync.dma_start(
                    e_v[:, sl],
                    e_in[q].rearrange("(p hb) w -> p (hb w)", hb=HB),
                ).then_inc(esem, 16)

            sync.dma_start(ct_v, c_in[:]).then_inc(csem, 16)
            sync.dma_start(id_s[:], i_in[:]).then_inc(csem, 16)
            for Q in range(PPC):
                loads(Q)
            if nloop > 1:
                r_v = sync.alloc_register("r_v")
                r_m = sync.alloc_register("r_m")
                sync.reg_mov(r_v, 3 * (PPC - NIO) + 3)
                sync.reg_mov(r_m, PPC - NIO + 1)
                with sync.Fori(0, nloop - 1):
                    for q in range(PPC):
                        loads(PPC + q, (r_v, r_m))
            sync.wait_ge(rsem, NQ)
            sync.dma_start(parts_out[:], parts_s[:]).then_inc(osem, 16)
            sync.wait_ge(osem, 16)

        # ---- ACT: G-copy, reduce -------------------------------------------
        @block.scalar
        def _(scalar):
            def pass_plane(Q, regs=None):
                # 1) G-copy(Q-1): PSUM -> SBUF bf16
                if Q >= 1:
                    if regs is None:
                        scalar.wait_ge(pesem, 3 * Q)
                    else:
                        r_pe, r_m = regs
                        scalar.wait_ge(pesem, r_pe)
                        scalar.reg_alu(r_pe, r_pe, 3, ADD)
                    nc.scalar.copy(
                        g_v[:, (Q - 1) % NSLOT], psum_f[:, (Q - 1) % NSLOT]
                    ).then_inc(gcsem, 1)
                # 2) reduce(Q-2)
                if Q >= 2:
                    if regs is None:
                        scalar.wait_ge(msem, Q - 1)
                    else:
                        scalar.wait_ge(msem, regs[1])
                        scalar.reg_alu(regs[1], regs[1], 1, ADD)
                    nc.scalar.activation(
                        junk_s[:], prod_v[:, (Q - 2) % NSLOT], COPY,
                        accum_out=parts_s[:, ((Q - 2) % PPC):((Q - 2) % PPC) + 1],
                    ).then_inc(rsem, 1)

            for Q in range(PPC):
                pass_plane(Q)
            if nloop > 1:
                r_pe = scalar.alloc_register("r_pe")
                r_m = scalar.alloc_register("r_m")
                scalar.reg_mov(r_pe, 3 * PPC)
                scalar.reg_mov(r_m, PPC - 1)
                with scalar.Fori(0, nloop - 1):
                    for q in range(PPC):
                        pass_plane(PPC + q, (r_pe, r_m))
            # drain: G-copy(NQ-1), reduce(NQ-2), reduce(NQ-1)
            scalar.wait_ge(pesem, 3 * NQ)
            nc.scalar.copy(
                g_v[:, (NQ - 1) % NSLOT], psum_f[:, (NQ - 1) % NSLOT]
            ).then_inc(gcsem, 1)
            scalar.wait_ge(msem, NQ - 1)
            nc.scalar.activation(
                junk_s[:], prod_v[:, (NQ - 2) % NSLOT], COPY,
                accum_out=parts_s[:, ((NQ - 2) % PPC):((NQ - 2) % PPC) + 1],
            ).then_inc(rsem, 1)
            scalar.wait_ge(msem, NQ)
            nc.scalar.activation(
                junk_s[:], prod_v[:, (NQ - 1) % NSLOT], COPY,
                accum_out=parts_s[:, ((NQ - 1) % PPC):((NQ - 1) % PPC) + 1],
            ).then_inc(rsem, 1)

        # ---- DVE: all 9 products + final mul -------------------------------
        @block.vector
        def _(vector):
            def pass_plane(Q, regs=None):
                ts = Q % NSLOT
                sl = Q % NIO
                q = Q % PPC
                if regs is None:
                    vector.wait_ge(dsem[q], 16)      # tf landed (tf-only sem)
                else:
                    r_d16, r_e, r_pe, r_gc = regs
                    vector.wait_ge(dsem[q], r_d16)
                for gi in (0, 1, 2):
                    g0, k, base, stride, _ = GROUPS[gi]
                    if regs is None:
                        if Q >= NSLOT:
                            vector.wait_ge(pesem, 3 * (Q - NSLOT) + gi + 1)
                    else:
                        vector.wait_ge(pesem, r_pe)
                        vector.reg_alu(r_pe, r_pe, 1, ADD)
                    nc.vector.tensor_tensor(
                        tmp_v[:, ts, g0:g0 + k],
                        ct_v[:, g0:g0 + k],
                        tf_ap(sl, base, stride, k),
                        MUL,
                    ).then_inc(vsem, 1)
                # final mul for plane Q-1
                if Q >= 1:
                    if regs is None:
                        vector.wait_ge(gcsem, Q)
                        vector.wait_ge(esem, 16 * Q)   # e(Q-1) landed
                    else:
                        vector.wait_ge(gcsem, r_gc)
                        vector.reg_alu(r_gc, r_gc, 1, ADD)
                        vector.wait_ge(esem, r_e)
                        vector.reg_alu(r_e, r_e, 16, ADD)
                    nc.vector.tensor_tensor(
                        prod_v[:, (Q - 1) % NSLOT],
                        e_v[:, (Q - 1) % NIO],
                        g_v[:, (Q - 1) % NSLOT],
                        MUL,
                    ).then_inc(msem, 1)

            vector.wait_ge(csem, 32)
            for Q in range(PPC):
                pass_plane(Q)
            if nloop > 1:
                r_d16 = vector.alloc_register("r_d16")
                r_e = vector.alloc_register("r_e")
                r_pe = vector.alloc_register("r_pe")
                r_gc = vector.alloc_register("r_gc")
                vector.reg_mov(r_d16, 32)
                vector.reg_mov(r_e, 16 * PPC)
                vector.reg_mov(r_pe, 3 * (PPC - NSLOT) + 1)
                vector.reg_mov(r_gc, PPC)
                with vector.Fori(0, nloop - 1):
                    for q in range(PPC):
                        pass_plane(PPC + q, (r_d16, r_e, r_pe, r_gc))
                    vector.reg_alu(r_d16, r_d16, 16, ADD)
            # drain: final mul for plane NQ-1
            vector.wait_ge(gcsem, NQ)
            nc.vector.tensor_tensor(
                prod_v[:, (NQ - 1) % NSLOT],
                e_v[:, (NQ - 1) % NIO],
                g_v[:, (NQ - 1) % NSLOT],
                MUL,
            ).then_inc(msem, 1)

        # ---- PE: identity-weight accumulating matmuls ----------------------
        @block.tensor
        def _(tensor):
            def mm_group(Q, gi, slots, regs=None):
                ts = Q % NSLOT
                if gi == 0:
                    # psum set reuse: G-copy(Q-NSLOT) done
                    if regs is None:
                        if Q >= NSLOT:
                            tensor.wait_ge(gcsem, Q - NSLOT + 1)
                    else:
                        tensor.wait_ge(gcsem, regs[1])
                        tensor.reg_alu(regs[1], regs[1], 1, ADD)
                if regs is None:
                    tensor.wait_ge(vsem, 3 * Q + gi + 1)
                else:
                    tensor.wait_ge(vsem, regs[0])
                    tensor.reg_alu(regs[0], regs[0], 1, ADD)
                n = len(slots)
                for j, s in enumerate(slots):
                    for b in range(HB):
                        mm = nc.tensor.matmul(
                            psum_v[:, ts, b],
                            id_s[:],
                            tmp_v[:, ts, s, b],
                            start=(s == 0),
                            stop=(s == 8),
                            skip_group_check=True,
                        )
                        if j == n - 1 and b == HB - 1:
                            mm.then_inc(pesem, 1)

            def pass_plane(Q, regs=None):
                for gi, slots in enumerate(PE_GROUPS):
                    mm_group(Q, gi, slots, regs)

            tensor.wait_ge(csem, 32)
            for Q in range(PPC):
                pass_plane(Q)
            if nloop > 1:
                r_v = tensor.alloc_register("r_v")
                r_gc = tensor.alloc_register("r_gc")
                tensor.reg_mov(r_v, 3 * PPC + 1)
                tensor.reg_mov(r_gc, PPC - NSLOT + 1)
                with tensor.Fori(0, nloop - 1):
                    for q in range(PPC):
                        pass_plane(PPC + q, (r_v, r_gc))
    return nc


def _get_nc(nloop=1):
    key = f"nc{nloop}"
    if key not in _CACHE:
        _CACHE[key] = _build_bass(nloop)
    return _CACHE[key]


def _make_runner(nc, in_maps):
    import time
    import jax
    import concourse.mybir as mybir
    from concourse import bass2jax
    from jax.sharding import Mesh, PartitionSpec, NamedSharding
    from jax.experimental.shard_map import shard_map

    pid = nc.partition_id_tensor.name if nc.partition_id_tensor else None
    in_names, out_names, out_avals, zeros = [], [], [], []
    for alloc in nc.m.functions[0].allocations:
        if type(alloc).__name__ != "MemoryLocationSet":
            continue
        name = alloc.memorylocations[0].name
        if alloc.kind == "ExternalInput":
            if name != pid:
                in_names.append(name)
        elif alloc.kind == "ExternalOutput":
            out_names.append(name)
            shape = tuple(alloc.tensor_shape)
            dt = mybir.dt.np(alloc.dtype)
            out_avals.append(jax.core.ShapedArray(shape, dt))
            zeros.append(np.zeros(shape, dt))
    n_params = len(in_names)
    all_names = in_names + out_names + ([pid] if pid else [])

    def _body(*args):
        ops = list(args)
        if pid:
            ops.append(bass2jax.partition_id_tensor())
        return tuple(bass2jax._bass_exec_p.bind(
            *ops, out_avals=tuple(out_avals), in_names=tuple(all_names),
            out_names=tuple(out_names), lowering_input_output_aliases=(),
            sim_require_finite=True, sim_require_nnan=True, nc=nc))

    devices = jax.devices()[:NCORES]
    mesh = Mesh(np.asarray(devices), ("core",))
    n_outs = len(out_names)
    sharded = jax.jit(
        shard_map(_body, mesh=mesh,
                  in_specs=(PartitionSpec("core"),) * (n_params + n_outs),
                  out_specs=(PartitionSpec("core"),) * n_outs,
                  check_rep=False),
        donate_argnums=tuple(range(n_params, n_params + n_outs)),
        keep_unused=True)
    sh = NamedSharding(mesh, PartitionSpec("core"))
    d_in = [jax.device_put(
                np.concatenate([np.asarray(m[k]) for m in in_maps], axis=0), sh)
            for k in in_names]
    cz = [np.concatenate([z] * NCORES, axis=0) for z in zeros]

    def run_once():
        dz = [jax.device_put(z, sh) for z in cz]
        for a in dz:
            a.block_until_ready()
        t0 = time.perf_counter()
        outs = sharded(*d_in, *dz)
        for o in outs:
            o.block_until_ready()
        return time.perf_counter() - t0
    return run_once


def benchmark(embeds, tf_embeds, raw, mask, iters=20, nloop=64):
    """Paired null/main timing of the hardware-looped program."""
    import concourse.bass as bass
    import concourse.mybir as mybir

    in_maps, _ = _prepare(embeds, tf_embeds, raw, mask)
    run_main = _make_runner(_get_nc(nloop), in_maps)

    f32 = mybir.dt.float32
    nc2 = bass.Bass()
    a_in = nc2.dram_tensor("a", [P, 16], f32, kind="ExternalInput")
    b_out = nc2.dram_tensor("b", [P, 16], f32, kind="ExternalOutput")
    with (nc2.sbuf_tensor([P, 16], f32) as t,
          nc2.semaphore() as s,
          nc2.semaphore() as o,
          nc2.Block() as blk):
        @blk.sync
        def _(sync):
            sync.dma_start(t[:], a_in[:]).then_inc(s, 16)
            sync.wait_ge(s, 16)
            sync.dma_start(b_out[:], t[:]).then_inc(o, 16)
            sync.wait_ge(o, 16)
    null_maps = [{"a": np.zeros((P, 16), np.float32)} for _ in range(NCORES)]
    run_null = _make_runner(nc2, null_maps)

    main_ts, null_ts = [], []
    for _ in range(iters):
        null_ts.append(run_null())
        main_ts.append(run_main())
    return main_ts, null_ts


def kernel(embeds, tf_embeds, raw, mask):
    from concourse.bass_utils import run_bass_kernel_spmd

    in_maps, csum = _prepare(embeds, tf_embeds, raw, mask)
    res = run_bass_kernel_spmd(
        _get_nc(1), in_maps, core_ids=list(range(NCORES)),
    )
    _CACHE["last_results"] = res

    s = np.float64(0.0)
    for om in res.results:
        s += om["partials"].astype(np.float64).sum()

    loss = (B * csum - s) / float(B * H * W)
    return np.asarray(loss, dtype=np.float32)
